# revision 1
# baseline (speedup 1.0000x reference)
"""GroupedQueryAttention Bass kernel for 8 Trainium2 NeuronCores.

Sharding: 8 devices = 2 batches x 4 sequence-quarters.
Device d handles batch b=d//4, query rows [512*i, 512*(i+1)) with i=d%4.

Per device:
  - K/V projection computed only for the local 512-row slice (+RoPE on K,
    V pre-transposed), then one AllGather over the 4 devices of the batch
    brings the full-sequence K^T and V to every device.
  - Q projection (all 16 heads) for the local slice, RoPE'd, overlaps the
    collective.
  - Attention runs in the transposed orientation: scores^T[sk,sq] chunks come
    straight from matmul(lhsT=k^T, rhs=q^T); exp on ScalarE (scale + per-head
    sink bias fused) writes P^T; out^T accumulates matmul(lhsT=v, rhs=P^T);
    softmax denominators accumulate via matmul(lhsT=ones).  Normalization is
    folded into the PSUM->SBUF drain.
  - o_proj consumes out^T directly as lhsT with streamed Wo; each device owns
    its full [512, 2048] output rows -> host just concatenates.

All matmuls use float32r (FP22 multiply, fp32 accumulate): full PE rate at
free-dim 512 with ~2e-4 relative error.

The softmax skips max-subtraction: logits are ~N(0, 2)-scaled values bounded
by ~+-30 for this problem family, far inside exp's fp32 range.  The additive
`sinks` bias per head is mathematically a softmax no-op but is still applied
(free, fused into the exp instruction).
"""

from contextlib import ExitStack

import numpy as np

import concourse.bass as bass
import concourse.tile as tile
from concourse import bacc, mybir
from concourse.bass_utils import run_bass_kernel_spmd
from concourse.masks import make_identity

F32 = mybir.dt.float32
F32R = mybir.dt.float32r
AF = mybir.ActivationFunctionType
ALU = mybir.AluOpType

# Problem dims (hardcoded per contract)
B = 2
S = 2048
E = 2048
HQ = 16
HKV = 4
D = 128
REP = HQ // HKV          # 4 q-heads per kv head
NDEV = 8
DPB = 4                  # devices per batch
SQ = S // DPB            # 512 local query rows
EC = E // 128            # 16 contraction chunks
SKC = S // 128           # 16 key chunks
SCALE = 1.0 / float(np.sqrt(D))

_CACHE = {}


def _build(sinks, with_bias_qkv, with_bias_o):
    nc = bacc.Bacc("TRN2", target_bir_lowering=False, debug=False, num_devices=NDEV)

    xT = nc.dram_tensor("xT", [E, SQ], F32R, kind="ExternalInput").ap()
    wq = nc.dram_tensor("wq", [E, HQ * D], F32R, kind="ExternalInput").ap()
    wk = nc.dram_tensor("wk", [E, HKV * D], F32R, kind="ExternalInput").ap()
    wv = nc.dram_tensor("wv", [E, HKV * D], F32R, kind="ExternalInput").ap()
    wo = nc.dram_tensor("wo", [HQ * D, E], F32R, kind="ExternalInput").ap()
    cosT = nc.dram_tensor("cosT", [D // 2, SQ], F32, kind="ExternalInput").ap()
    sinT = nc.dram_tensor("sinT", [D // 2, SQ], F32, kind="ExternalInput").ap()
    if with_bias_qkv:
        # laid out [D, H] so a column is the per-partition bias of one head
        bqd = nc.dram_tensor("bqd", [D, HQ], F32, kind="ExternalInput").ap()
        bkd = nc.dram_tensor("bkd", [D, HKV], F32, kind="ExternalInput").ap()
        bvd = nc.dram_tensor("bvd", [D, HKV], F32, kind="ExternalInput").ap()
    if with_bias_o:
        bod = nc.dram_tensor("bod", [1, E], F32, kind="ExternalInput").ap()
    out = nc.dram_tensor("out", [SQ, E], F32, kind="ExternalOutput").ap()

    with tile.TileContext(nc) as tc, ExitStack() as es:
        _emit(tc, es, locals(), sinks, with_bias_qkv, with_bias_o)
    nc.compile()
    return nc


def _emit(tc, es, t, sinks, with_bias_qkv, with_bias_o):
    nc = tc.nc
    xT, wq, wk, wv, wo = t["xT"], t["wq"], t["wk"], t["wv"], t["wo"]
    cosT, sinT, out = t["cosT"], t["sinT"], t["out"]

    # ---------- persistent pools ----------
    const_pool = es.enter_context(tc.tile_pool(name="const", bufs=1))
    dram = es.enter_context(tc.tile_pool(name="dram", bufs=1, space="DRAM"))

    ident_f = const_pool.tile([128, 128], F32, tag="ident_f")
    make_identity(nc, ident_f[:])
    ident = const_pool.tile([128, 128], F32R, tag="ident")
    nc.vector.tensor_copy(ident[:], ident_f[:])
    ones_f = const_pool.tile([128, 1], F32, tag="ones_f")
    nc.vector.memset(ones_f[:], 1.0)
    ones = const_pool.tile([128, 1], F32R, tag="ones")
    nc.vector.tensor_copy(ones[:], ones_f[:])

    if with_bias_qkv:
        bq_sb = const_pool.tile([D, HQ], F32, tag="bq")
        nc.sync.dma_start(bq_sb[:], t["bqd"])
        bk_sb = const_pool.tile([D, HKV], F32, tag="bk")
        nc.sync.dma_start(bk_sb[:], t["bkd"])
        bv_sb = const_pool.tile([D, HKV], F32, tag="bv")
        nc.sync.dma_start(bv_sb[:], t["bvd"])

    sinks_sb = const_pool.tile([128, HQ], F32, tag="sinks")
    for _h in range(HQ):
        nc.vector.memset(sinks_sb[:, _h : _h + 1], float(sinks[_h]))

    kv_slice = dram.tile([2, 4 * D, SQ], F32R, tag="kvs")   # [0]=k^T slice, [1]=v slice (s-major)
    kv_gath = dram.tile([DPB, 2, 4 * D, SQ], F32R, tag="kvg")


    def rope(dst, src_ps, n_heads, cos_t, sin_t, tmp_pool, bias_sb=None, head0=0):
        """dst/src: [128, n_heads*SQ]; halves along partitions. bias optional."""
        w = n_heads * SQ
        src = src_ps[:].rearrange("p (h s) -> p h s", h=n_heads)
        if bias_sb is not None:
            # add per-(head,d) bias before rotation, head-by-head
            for j in range(n_heads):
                nc.vector.tensor_scalar_add(
                    src_ps[:, j * SQ : (j + 1) * SQ],
                    src_ps[:, j * SQ : (j + 1) * SQ],
                    bias_sb[:, head0 + j : head0 + j + 1],
                )
        dstv = dst[:].rearrange("p (h s) -> p h s", h=n_heads)
        cosb = cos_t[:, None, :].to_broadcast((64, n_heads, SQ))
        sinb = sin_t[:, None, :].to_broadcast((64, n_heads, SQ))
        q1 = src[0:64]
        q2 = src[64:128]
        m1 = tmp_pool.tile([64, w], F32, tag="m", name="m1")[:].rearrange("p (h s) -> p h s", h=n_heads)
        m2 = tmp_pool.tile([64, w], F32, tag="m", name="m2")[:].rearrange("p (h s) -> p h s", h=n_heads)
        nc.vector.tensor_tensor(m1, q1, cosb, ALU.mult)
        nc.vector.tensor_tensor(m2, q2, sinb, ALU.mult)
        nc.vector.tensor_tensor(dstv[0:64], m1, m2, ALU.subtract)
        m3 = tmp_pool.tile([64, w], F32, tag="m", name="m3")[:].rearrange("p (h s) -> p h s", h=n_heads)
        m4 = tmp_pool.tile([64, w], F32, tag="m", name="m4")[:].rearrange("p (h s) -> p h s", h=n_heads)
        nc.vector.tensor_tensor(m3, q2, cosb, ALU.mult)
        nc.vector.tensor_tensor(m4, q1, sinb, ALU.mult)
        nc.vector.tensor_tensor(dstv[64:128], m3, m4, ALU.add)

    # ---------- phase 1: local KV projection + rope + transpose + gather ----
    with (
        tc.tile_pool(name="p12", bufs=1) as p12,
        tc.tile_pool(name="wkv", bufs=24) as wkv_pool,
        tc.tile_pool(name="proj_ps", bufs=3, space="PSUM") as proj_ps,
        tc.tile_pool(name="tr_ps", bufs=2, space="PSUM") as tr_ps,
        tc.tile_pool(name="rope_tmp", bufs=4) as rope_tmp,
        tc.tile_pool(name="kvout", bufs=2) as kvout,
        tc.tile_pool(name="vtr", bufs=4) as vtr,
    ):
        xT_sb = p12.tile([128, EC * SQ], F32R, tag="xT")
        nc.sync.dma_start(
            xT_sb[:].rearrange("p (c s) -> p c s", s=SQ),
            xT.rearrange("(c p) s -> p c s", p=128),
        )
        xview = xT_sb[:].rearrange("p (c s) -> p c s", s=SQ)
        cos_sb = p12.tile([64, SQ], F32, tag="cos")
        nc.sync.dma_start(cos_sb[:], cosT)
        sin_sb = p12.tile([64, SQ], F32, tag="sin")
        nc.sync.dma_start(sin_sb[:], sinT)

        # K and V: 4 kv heads each, grouped 2 heads per psum tile
        kv_sb = {}
        for which, w_dram, bias in (
            ("k", wk, "bk"),
            ("v", wv, "bv"),
        ):
            sb = kvout.tile([128, HKV * SQ], F32R, tag=f"{which}_sb")
            kv_sb[which] = sb
            for g in range(HKV // 2):   # 2 heads per group
                ps = proj_ps.tile([128, 2 * SQ], F32, tag="proj")
                for j in range(2):
                    h = g * 2 + j
                    for c in range(EC):
                        wt = wkv_pool.tile([128, 128], F32R, tag="wchunk")
                        nc.sync.dma_start(wt[:], w_dram[c * 128 : (c + 1) * 128, h * 128 : (h + 1) * 128])
                        nc.tensor.matmul(
                            ps[:, j * SQ : (j + 1) * SQ],
                            wt[:],
                            xview[:, c, :],
                            start=(c == 0),
                            stop=(c == EC - 1),
                        )
                dst = sb[:, g * 2 * SQ : (g + 1) * 2 * SQ].rearrange("p (h s) -> p h s", h=2)
                if which == "k":
                    rope(
                        sb[:, g * 2 * SQ : (g + 1) * 2 * SQ],
                        ps, 2, cos_sb, sin_sb, rope_tmp,
                        bias_sb=(bk_sb if with_bias_qkv else None), head0=g * 2,
                    )
                else:
                    if with_bias_qkv:
                        for j in range(2):
                            nc.vector.tensor_scalar_add(
                                ps[:, j * SQ : (j + 1) * SQ],
                                ps[:, j * SQ : (j + 1) * SQ],
                                bv_sb[:, g * 2 + j : g * 2 + j + 1],
                            )
                    nc.vector.tensor_copy(sb[:, g * 2 * SQ : (g + 1) * 2 * SQ], ps[:])

        # k^T slice out: head h -> kv_slice[0, h*128:(h+1)*128, :]
        for h in range(HKV):
            nc.sync.dma_start(
                kv_slice[0, h * 128 : (h + 1) * 128, :],
                kv_sb["k"][:, h * SQ : (h + 1) * SQ],
            )
        # v: transpose [d, s-block] -> [s-block, d], write s-major slice
        for h in range(HKV):
            for sc in range(SQ // 128):
                tp = tr_ps.tile([128, 128], F32R, tag="trp")
                nc.tensor.transpose(
                    tp[:], kv_sb["v"][:, h * SQ + sc * 128 : h * SQ + (sc + 1) * 128], ident[:]
                )
                ts_ = vtr.tile([128, 128], F32R, tag="vts")
                nc.vector.tensor_copy(ts_[:], tp[:])
                nc.sync.dma_start(
                    kv_slice[1, sc * 128 : (sc + 1) * 128, h * 128 : (h + 1) * 128],
                    ts_[:],
                )

        nc.gpsimd.collective_compute(
            "AllGather",
            ALU.bypass,
            ins=[kv_slice[:].opt()],
            outs=[kv_gath[:].opt()],
            replica_groups=[[0, 1, 2, 3], [4, 5, 6, 7]],
        )

        # ---------- phase 2: Q projection + rope (overlaps collective) ------
        q_sb = const_pool.tile([128, HQ * SQ], F32R, tag="q_sb")
        with tc.tile_pool(name="wq_pool", bufs=24) as wq_pool:
            for g in range(HQ // 2):
                ps = proj_ps.tile([128, 2 * SQ], F32, tag="proj")
                for j in range(2):
                    h = g * 2 + j
                    for c in range(EC):
                        wt = wq_pool.tile([128, 128], F32R, tag="wqchunk")
                        nc.sync.dma_start(wt[:], wq[c * 128 : (c + 1) * 128, h * 128 : (h + 1) * 128])
                        nc.tensor.matmul(
                            ps[:, j * SQ : (j + 1) * SQ],
                            wt[:],
                            xview[:, c, :],
                            start=(c == 0),
                            stop=(c == EC - 1),
                        )
                rope(
                    q_sb[:, g * 2 * SQ : (g + 1) * 2 * SQ],
                    ps, 2, cos_sb, sin_sb, rope_tmp,
                    bias_sb=(bq_sb if with_bias_qkv else None), head0=g * 2,
                )

    # ---------- phase 3: attention ----------
    attn_sb = const_pool.tile([128, HQ * SQ], F32R, tag="attn_sb")  # out^T per head

    with (
        tc.tile_pool(name="kv_all", bufs=1) as kv_all,
        tc.tile_pool(name="wo_pool", bufs=3) as wo_pool,
        ExitStack() as attn_es,
    ):
        sc_ps = attn_es.enter_context(tc.tile_pool(name="sc_ps", bufs=3, space="PSUM"))
        out_ps = attn_es.enter_context(tc.tile_pool(name="out_ps", bufs=2, space="PSUM"))
        sum_ps = attn_es.enter_context(tc.tile_pool(name="sum_ps", bufs=2, space="PSUM"))
        p_pool = attn_es.enter_context(tc.tile_pool(name="p_pool", bufs=4))
        den_pool = attn_es.enter_context(tc.tile_pool(name="den_pool", bufs=3))
        # full-sequence K^T and V per kv head
        k_all = kv_all.tile([128, HKV * S], F32R, tag="k_all")   # [d, h*S + sk]
        v_all = kv_all.tile([128, HKV * S], F32R, tag="v_all")   # [s%128, h*S + c*128 + d]
        for h in range(HKV):
            for si in range(DPB):
                nc.sync.dma_start(
                    k_all[:, h * S + si * SQ : h * S + (si + 1) * SQ],
                    kv_gath[si, 0, h * 128 : (h + 1) * 128, :],
                )
                for sc in range(SQ // 128):
                    c = si * (SQ // 128) + sc
                    nc.sync.dma_start(
                        v_all[:, h * S + c * 128 : h * S + (c + 1) * 128],
                        kv_gath[si, 1, sc * 128 : (sc + 1) * 128, h * 128 : (h + 1) * 128],
                    )


        for h in range(HQ):
            kh = h // REP
            op = out_ps.tile([128, SQ], F32, tag="outp")
            sp = sum_ps.tile([1, SQ], F32, tag="sump")
            for c in range(SKC):
                scp = sc_ps.tile([128, SQ], F32, tag="scp")
                nc.tensor.matmul(
                    scp[:],
                    k_all[:, kh * S + c * 128 : kh * S + (c + 1) * 128],
                    q_sb[:, h * SQ : (h + 1) * SQ],
                    start=True,
                    stop=True,
                )
                pt = p_pool.tile([128, SQ], F32R, tag="pt")
                nc.scalar.activation(pt[:], scp[:], AF.Exp, bias=sinks_sb[:, h : h + 1], scale=SCALE)
                nc.tensor.matmul(
                    op[:],
                    v_all[:, kh * S + c * 128 : kh * S + (c + 1) * 128],
                    pt[:],
                    start=(c == 0),
                    stop=(c == SKC - 1),
                    skip_group_check=True,
                )
                nc.tensor.matmul(
                    sp[:],
                    ones[:],
                    pt[:],
                    start=(c == 0),
                    stop=(c == SKC - 1),
                    skip_group_check=True,
                )
            rs = den_pool.tile([1, SQ], F32, tag="rs")
            nc.vector.reciprocal(rs[:], sp[:])
            den = den_pool.tile([128, SQ], F32, tag="den")
            nc.gpsimd.partition_broadcast(den[:], rs[:])
            nc.vector.tensor_tensor(
                attn_sb[:, h * SQ : (h + 1) * SQ], op[:], den[:], ALU.mult
            )

        # ---------- phase 4: o_proj ----------
        attn_es.close()
        with (
            tc.tile_pool(name="o_ps", bufs=2, space="PSUM") as o_ps,
            tc.tile_pool(name="o_sb", bufs=3) as o_sb_pool,
        ):
            if with_bias_o:
                bo_sb = const_pool.tile([1, E], F32, tag="bo")
                nc.sync.dma_start(bo_sb[:], t["bod"])
                bo_b = const_pool.tile([128, E], F32, tag="bo_b")
                nc.gpsimd.partition_broadcast(bo_b[:], bo_sb[:])
            for et in range(4):
                wo_halves = []
                for half in range(2):
                    wt = wo_pool.tile([128, (EC // 2) * 512], F32R, tag="wo_half",
                                      name=f"wo_{et}_{half}")
                    nc.sync.dma_start(
                        wt[:].rearrange("p (c n) -> p c n", n=512),
                        wo.rearrange("(c p) e -> p c e", p=128)[
                            :, half * (EC // 2) : (half + 1) * (EC // 2),
                            et * 512 : (et + 1) * 512,
                        ],
                    )
                    wo_halves.append(wt[:].rearrange("p (c n) -> p c n", n=512))
                for sqc in range(SQ // 128):
                    ps = o_ps.tile([128, 512], F32, tag="ops")
                    for hd in range(HQ):
                        nc.tensor.matmul(
                            ps[:],
                            attn_sb[:, hd * SQ + sqc * 128 : hd * SQ + (sqc + 1) * 128],
                            wo_halves[hd // (EC // 2)][:, hd % (EC // 2), :],
                            start=(hd == 0),
                            stop=(hd == HQ - 1),
                        )
                    ot = o_sb_pool.tile([128, 512], F32, tag="osb")
                    if with_bias_o:
                        nc.vector.tensor_tensor(
                            ot[:], ps[:], bo_b[:, et * 512 : (et + 1) * 512], ALU.add
                        )
                    else:
                        nc.scalar.copy(ot[:], ps[:])
                    nc.sync.dma_start(
                        out[sqc * 128 : (sqc + 1) * 128, et * 512 : (et + 1) * 512],
                        ot[:],
                    )


RUN_KWARGS = {}


def kernel(x, sin, cos, Wq, bq, Wk, bk, Wv, bv, Wo, bo, sinks):
    x = np.asarray(x, dtype=np.float32)
    sin = np.asarray(sin, dtype=np.float32)
    cos = np.asarray(cos, dtype=np.float32)
    sinks = np.asarray(sinks, dtype=np.float32)
    with_bias_qkv = bool(np.any(bq) or np.any(bk) or np.any(bv))
    with_bias_o = bool(np.any(bo))

    key = (sinks.tobytes(), with_bias_qkv, with_bias_o)
    if key not in _CACHE:
        _CACHE[key] = _build(sinks, with_bias_qkv, with_bias_o)
    nc = _CACHE[key]

    wq_f = np.ascontiguousarray(Wq, dtype=np.float32)
    wk_f = np.ascontiguousarray(Wk, dtype=np.float32)
    wv_f = np.ascontiguousarray(Wv, dtype=np.float32)
    wo_f = np.ascontiguousarray(Wo, dtype=np.float32)

    in_maps = []
    for dev in range(NDEV):
        b, i = divmod(dev, DPB)
        sl = slice(SQ * i, SQ * (i + 1))
        m = {
            "xT": np.ascontiguousarray(x[b, sl, :].T),
            "wq": wq_f,
            "wk": wk_f,
            "wv": wv_f,
            "wo": wo_f,
            "cosT": np.ascontiguousarray(cos[b, sl, :].T),
            "sinT": np.ascontiguousarray(sin[b, sl, :].T),
        }
        if with_bias_qkv:
            m["bqd"] = np.ascontiguousarray(np.asarray(bq, np.float32).reshape(HQ, D).T)
            m["bkd"] = np.ascontiguousarray(np.asarray(bk, np.float32).reshape(HKV, D).T)
            m["bvd"] = np.ascontiguousarray(np.asarray(bv, np.float32).reshape(HKV, D).T)
        if with_bias_o:
            m["bod"] = np.asarray(bo, np.float32).reshape(1, E)
        in_maps.append(m)

    res = run_bass_kernel_spmd(nc, in_maps, list(range(NDEV)), **RUN_KWARGS)
    kernel.last_result = res

    out = np.empty((B, S, E), dtype=np.float32)
    for dev in range(NDEV):
        b, i = divmod(dev, DPB)
        out[b, SQ * i : SQ * (i + 1), :] = res.results[dev]["out"]
    return out



# revision 4
# speedup vs baseline: 1.4962x; 1.4962x over previous
"""GroupedQueryAttention Bass kernel for 8 Trainium2 NeuronCores.

Sharding: 8 devices = 2 batches x 4 sequence-quarters.
Device d handles batch b=d//4, query rows [512*i, 512*(i+1)) with i=d%4.

v2: all matmul operands in BF16 (enables FastWeightLoad -> dense PE stream
that keeps the HAM clock gate at 2.4GHz; the fp32r baseline ran the PE cold
at 1.2GHz for 93% of the kernel).  Weights are pre-tiled on the host into
contiguous per-partition layouts so each weight is 1-4 large DMAs instead of
hundreds of strided 64KB ones.  Exp is batched 2 k-chunks per activation
instruction, and the attention inner loop is software-pipelined (scores for
group cg issue before AV/ones of group cg-1) so the PE FIFO never
head-of-line blocks on ScalarE's exp.

Per device:
  - K/V projection for the local 512-row slice (+RoPE on K, V transposed to
    s-major), AllGather over the 4 devices of the batch -> full-sequence K^T
    and V.  Q projection (16 heads) overlaps the collective.
  - Attention in transposed orientation: scores^T chunks from
    matmul(lhsT=k^T, rhs=q^T); exp on ScalarE (scale + sink bias fused);
    out^T accumulates matmul(lhsT=v, rhs=P^T); softmax denominators via
    matmul(lhsT=ones); normalization folded into the PSUM drain.
  - o_proj consumes out^T as lhsT with resident Wo; each device owns its
    [512, 2048] output rows -> host concatenates.
"""

from contextlib import ExitStack

import numpy as np
from ml_dtypes import bfloat16

import concourse.bass as bass
import concourse.tile as tile
from concourse import bacc, mybir
from concourse.bass_utils import run_bass_kernel_spmd
from concourse.masks import make_identity

F32 = mybir.dt.float32
BF16 = mybir.dt.bfloat16
AF = mybir.ActivationFunctionType
ALU = mybir.AluOpType

# Problem dims (hardcoded per contract)
B = 2
S = 2048
E = 2048
HQ = 16
HKV = 4
D = 128
REP = HQ // HKV          # 4 q-heads per kv head
NDEV = 8
DPB = 4                  # devices per batch
SQ = S // DPB            # 512 local query rows
EC = E // 128            # 16 contraction chunks
SKC = S // 128           # 16 key chunks
GC = 2                   # k-chunks per exp group
NG = SKC // GC           # 8 exp groups per head
SCALE = 1.0 / float(np.sqrt(D))

_CACHE = {}


def _build(sinks, with_bias_qkv, with_bias_o):
    nc = bacc.Bacc("TRN2", target_bir_lowering=False, debug=False, num_devices=NDEV)

    xT = nc.dram_tensor("xT", [128, EC * SQ], BF16, kind="ExternalInput").ap()
    wq = nc.dram_tensor("wq", [128, HQ * EC * 128], BF16, kind="ExternalInput").ap()
    wk = nc.dram_tensor("wk", [128, HKV * EC * 128], BF16, kind="ExternalInput").ap()
    wv = nc.dram_tensor("wv", [128, HKV * EC * 128], BF16, kind="ExternalInput").ap()
    wo = nc.dram_tensor("wo", [128, HQ * E], BF16, kind="ExternalInput").ap()
    cosT = nc.dram_tensor("cosT", [D // 2, SQ], F32, kind="ExternalInput").ap()
    sinT = nc.dram_tensor("sinT", [D // 2, SQ], F32, kind="ExternalInput").ap()
    if with_bias_qkv:
        # laid out [D, H] so a column is the per-partition bias of one head
        bqd = nc.dram_tensor("bqd", [D, HQ], F32, kind="ExternalInput").ap()
        bkd = nc.dram_tensor("bkd", [D, HKV], F32, kind="ExternalInput").ap()
        bvd = nc.dram_tensor("bvd", [D, HKV], F32, kind="ExternalInput").ap()
    if with_bias_o:
        bod = nc.dram_tensor("bod", [1, E], F32, kind="ExternalInput").ap()
    out = nc.dram_tensor("out", [SQ, E], F32, kind="ExternalOutput").ap()

    with tile.TileContext(nc) as tc, ExitStack() as es:
        _emit(tc, es, locals(), sinks, with_bias_qkv, with_bias_o)
    nc.compile()
    return nc


def _emit(tc, es, t, sinks, with_bias_qkv, with_bias_o):
    nc = tc.nc
    xT, wq, wk, wv, wo = t["xT"], t["wq"], t["wk"], t["wv"], t["wo"]
    cosT, sinT, out = t["cosT"], t["sinT"], t["out"]

    # ---------- persistent pools ----------
    const_pool = es.enter_context(tc.tile_pool(name="const", bufs=1))
    dram = es.enter_context(tc.tile_pool(name="dram", bufs=1, space="DRAM"))

    ident_f = const_pool.tile([128, 128], F32, tag="ident_f")
    make_identity(nc, ident_f[:])
    ident = const_pool.tile([128, 128], BF16, tag="ident")
    nc.vector.tensor_copy(ident[:], ident_f[:])
    ones_f = const_pool.tile([128, 1], F32, tag="ones_f")
    nc.vector.memset(ones_f[:], 1.0)
    ones = const_pool.tile([128, 1], BF16, tag="ones")
    nc.vector.tensor_copy(ones[:], ones_f[:])

    if with_bias_qkv:
        bq_sb = const_pool.tile([D, HQ], F32, tag="bq")
        nc.sync.dma_start(bq_sb[:], t["bqd"])
        bk_sb = const_pool.tile([D, HKV], F32, tag="bk")
        nc.sync.dma_start(bk_sb[:], t["bkd"])
        bv_sb = const_pool.tile([D, HKV], F32, tag="bv")
        nc.sync.dma_start(bv_sb[:], t["bvd"])

    sinks_sb = const_pool.tile([128, HQ], F32, tag="sinks")
    for _h in range(HQ):
        nc.vector.memset(sinks_sb[:, _h : _h + 1], float(sinks[_h]))

    q_sb = const_pool.tile([128, HQ * SQ], BF16, tag="q_sb")        # q^T, rope'd
    attn_sb = const_pool.tile([128, HQ * SQ], BF16, tag="attn_sb")  # out^T per head

    kv_slice = dram.tile([2, 4 * D, SQ], BF16, tag="kvs")   # [0]=k^T slice, [1]=v slice (s-major)
    kv_gath = dram.tile([DPB, 2, 4 * D, SQ], BF16, tag="kvg")

    def rope(dst, src_ps, n_heads, cos_t, sin_t, tmp_pool, bias_sb=None, head0=0):
        """dst/src: [128, n_heads*SQ]; halves along partitions. bias optional."""
        w = n_heads * SQ
        src = src_ps[:].rearrange("p (h s) -> p h s", h=n_heads)
        if bias_sb is not None:
            for j in range(n_heads):
                nc.vector.tensor_scalar_add(
                    src_ps[:, j * SQ : (j + 1) * SQ],
                    src_ps[:, j * SQ : (j + 1) * SQ],
                    bias_sb[:, head0 + j : head0 + j + 1],
                )
        dstv = dst[:].rearrange("p (h s) -> p h s", h=n_heads)
        cosb = cos_t[:, None, :].to_broadcast((64, n_heads, SQ))
        sinb = sin_t[:, None, :].to_broadcast((64, n_heads, SQ))
        q1 = src[0:64]
        q2 = src[64:128]
        m1 = tmp_pool.tile([64, w], F32, tag="m", name="m1")[:].rearrange("p (h s) -> p h s", h=n_heads)
        m2 = tmp_pool.tile([64, w], F32, tag="m", name="m2")[:].rearrange("p (h s) -> p h s", h=n_heads)
        nc.vector.tensor_tensor(m1, q1, cosb, ALU.mult)
        nc.vector.tensor_tensor(m2, q2, sinb, ALU.mult)
        nc.vector.tensor_tensor(dstv[0:64], m1, m2, ALU.subtract)
        m3 = tmp_pool.tile([64, w], F32, tag="m", name="m3")[:].rearrange("p (h s) -> p h s", h=n_heads)
        m4 = tmp_pool.tile([64, w], F32, tag="m", name="m4")[:].rearrange("p (h s) -> p h s", h=n_heads)
        nc.vector.tensor_tensor(m3, q2, cosb, ALU.mult)
        nc.vector.tensor_tensor(m4, q1, sinb, ALU.mult)
        nc.vector.tensor_tensor(dstv[64:128], m3, m4, ALU.add)

    # ---------- phase 1: local KV projection + rope + transpose + gather ----
    with (
        tc.tile_pool(name="p12", bufs=1) as p12,
        tc.tile_pool(name="proj_ps", bufs=3, space="PSUM") as proj_ps,
        tc.tile_pool(name="tr_ps", bufs=2, space="PSUM") as tr_ps,
        tc.tile_pool(name="rope_tmp", bufs=4) as rope_tmp,
        tc.tile_pool(name="vtr", bufs=4) as vtr,
    ):
        xT_sb = p12.tile([128, EC * SQ], BF16, tag="xT")
        nc.sync.dma_start(xT_sb[:], xT)
        xview = xT_sb[:].rearrange("p (c s) -> p c s", s=SQ)
        cos_sb = p12.tile([64, SQ], F32, tag="cos")
        nc.sync.dma_start(cos_sb[:], cosT)
        sin_sb = p12.tile([64, SQ], F32, tag="sin")
        nc.sync.dma_start(sin_sb[:], sinT)

        wk_sb = p12.tile([128, HKV * EC * 128], BF16, tag="wk")
        nc.sync.dma_start(wk_sb[:], wk)
        wv_sb = p12.tile([128, HKV * EC * 128], BF16, tag="wv")
        nc.sync.dma_start(wv_sb[:], wv)
        wkview = wk_sb[:].rearrange("p (h c n) -> p h c n", c=EC, n=128)
        wvview = wv_sb[:].rearrange("p (h c n) -> p h c n", c=EC, n=128)

        # K and V: 4 kv heads each, grouped 2 heads per psum tile
        kv_sb = {}
        for which, wview, bias in (
            ("k", wkview, "bk"),
            ("v", wvview, "bv"),
        ):
            sb = p12.tile([128, HKV * SQ], BF16, tag=f"{which}_sb")
            kv_sb[which] = sb
            for g in range(HKV // 2):   # 2 heads per group
                ps = proj_ps.tile([128, 2 * SQ], F32, tag="proj")
                for j in range(2):
                    h = g * 2 + j
                    for c in range(EC):
                        nc.tensor.matmul(
                            ps[:, j * SQ : (j + 1) * SQ],
                            wview[:, h, c, :],
                            xview[:, c, :],
                            start=(c == 0),
                            stop=(c == EC - 1),
                        )
                if which == "k":
                    rope(
                        sb[:, g * 2 * SQ : (g + 1) * 2 * SQ],
                        ps, 2, cos_sb, sin_sb, rope_tmp,
                        bias_sb=(bk_sb if with_bias_qkv else None), head0=g * 2,
                    )
                else:
                    if with_bias_qkv:
                        for j in range(2):
                            nc.vector.tensor_scalar_add(
                                ps[:, j * SQ : (j + 1) * SQ],
                                ps[:, j * SQ : (j + 1) * SQ],
                                bv_sb[:, g * 2 + j : g * 2 + j + 1],
                            )
                    nc.vector.tensor_copy(sb[:, g * 2 * SQ : (g + 1) * 2 * SQ], ps[:])

        # k^T slice out: head h -> kv_slice[0, h*128:(h+1)*128, :]
        for h in range(HKV):
            nc.sync.dma_start(
                kv_slice[0, h * 128 : (h + 1) * 128, :],
                kv_sb["k"][:, h * SQ : (h + 1) * SQ],
            )
        # v: transpose [d, s-block] -> [s-block, d], write s-major slice
        for h in range(HKV):
            for sc in range(SQ // 128):
                tp = tr_ps.tile([128, 128], BF16, tag="trp")
                nc.tensor.transpose(
                    tp[:], kv_sb["v"][:, h * SQ + sc * 128 : h * SQ + (sc + 1) * 128], ident[:]
                )
                ts_ = vtr.tile([128, 128], BF16, tag="vts")
                nc.vector.tensor_copy(ts_[:], tp[:])
                nc.sync.dma_start(
                    kv_slice[1, sc * 128 : (sc + 1) * 128, h * 128 : (h + 1) * 128],
                    ts_[:],
                )

        nc.gpsimd.collective_compute(
            "AllGather",
            ALU.bypass,
            ins=[kv_slice[:].opt()],
            outs=[kv_gath[:].opt()],
            replica_groups=[[0, 1, 2, 3], [4, 5, 6, 7]],
        )

        # ---------- phase 2: Q projection + rope (overlaps collective) ------
        wq_sb = p12.tile([128, HQ * EC * 128], BF16, tag="wq")
        QW = 4 * EC * 128  # 4 heads per DMA
        for g4 in range(HQ // 4):
            nc.sync.dma_start(
                wq_sb[:, g4 * QW : (g4 + 1) * QW], wq[:, g4 * QW : (g4 + 1) * QW]
            )
        wqview = wq_sb[:].rearrange("p (h c n) -> p h c n", c=EC, n=128)
        for g in range(HQ // 2):
            ps = proj_ps.tile([128, 2 * SQ], F32, tag="proj")
            for j in range(2):
                h = g * 2 + j
                for c in range(EC):
                    nc.tensor.matmul(
                        ps[:, j * SQ : (j + 1) * SQ],
                        wqview[:, h, c, :],
                        xview[:, c, :],
                        start=(c == 0),
                        stop=(c == EC - 1),
                    )
            rope(
                q_sb[:, g * 2 * SQ : (g + 1) * 2 * SQ],
                ps, 2, cos_sb, sin_sb, rope_tmp,
                bias_sb=(bq_sb if with_bias_qkv else None), head0=g * 2,
            )

    # ---------- phase 3: attention ----------
    with (
        tc.tile_pool(name="kv_all", bufs=1) as kv_all,
        tc.tile_pool(name="wo_pool", bufs=1) as wo_pool,
        ExitStack() as attn_es,
    ):
        # Wo resident for phase 4; DMA streams during the collective wait.
        wo_sb = wo_pool.tile([128, HQ * E], BF16, tag="wo_sb")
        OW = 4 * E
        for g4 in range(HQ // 4):
            nc.sync.dma_start(
                wo_sb[:, g4 * OW : (g4 + 1) * OW], wo[:, g4 * OW : (g4 + 1) * OW]
            )
        woview = wo_sb[:].rearrange("p (h e) -> p h e", e=E)

        sc_ps = attn_es.enter_context(tc.tile_pool(name="sc_ps", bufs=2, space="PSUM"))
        out_ps = attn_es.enter_context(tc.tile_pool(name="out_ps", bufs=2, space="PSUM"))
        sum_ps = attn_es.enter_context(tc.tile_pool(name="sum_ps", bufs=2, space="PSUM"))
        p_pool = attn_es.enter_context(tc.tile_pool(name="p_pool", bufs=4))
        den_pool = attn_es.enter_context(tc.tile_pool(name="den_pool", bufs=3))

        # full-sequence K^T and V per kv head
        k_all = kv_all.tile([128, HKV * S], BF16, tag="k_all")   # [d, h*S + sk]
        v_all = kv_all.tile([128, HKV * S], BF16, tag="v_all")   # [s%128, h*S + c*128 + d]
        for h in range(HKV):
            for si in range(DPB):
                nc.sync.dma_start(
                    k_all[:, h * S + si * SQ : h * S + (si + 1) * SQ],
                    kv_gath[si, 0, h * 128 : (h + 1) * 128, :],
                )
                nc.sync.dma_start(
                    v_all[:, h * S + si * SQ : h * S + (si + 1) * SQ].rearrange(
                        "p (c d) -> p c d", d=128
                    ),
                    kv_gath[si, 1, :, h * 128 : (h + 1) * 128].rearrange(
                        "(c p) d -> p c d", p=128
                    ),
                )

        for h in range(HQ):
            kh = h // REP
            op = out_ps.tile([128, SQ], F32, tag="outp")
            sp = sum_ps.tile([1, SQ], F32, tag="sump")
            pts = [None] * NG

            def emit_av(g):
                for j in range(GC):
                    c = g * GC + j
                    nc.tensor.matmul(
                        op[:],
                        v_all[:, kh * S + c * 128 : kh * S + (c + 1) * 128],
                        pts[g][:, j * 512 : (j + 1) * 512],
                        start=(c == 0),
                        stop=(c == SKC - 1),
                        skip_group_check=True,
                    )
                    nc.tensor.matmul(
                        sp[:],
                        ones[:],
                        pts[g][:, j * 512 : (j + 1) * 512],
                        start=(c == 0),
                        stop=(c == SKC - 1),
                        skip_group_check=True,
                    )

            for cg in range(NG):
                scp = sc_ps.tile([128, GC * 512], F32, tag="scp")
                for j in range(GC):
                    c = cg * GC + j
                    nc.tensor.matmul(
                        scp[:, j * 512 : (j + 1) * 512],
                        k_all[:, kh * S + c * 128 : kh * S + (c + 1) * 128],
                        q_sb[:, h * SQ : (h + 1) * SQ],
                        start=True,
                        stop=True,
                    )
                pt = p_pool.tile([128, GC * 512], BF16, tag="pt")
                nc.scalar.activation(pt[:], scp[:], AF.Exp, bias=sinks_sb[:, h : h + 1], scale=SCALE)
                pts[cg] = pt
                if cg >= 1:
                    emit_av(cg - 1)
            emit_av(NG - 1)

            rs = den_pool.tile([1, SQ], F32, tag="rs")
            nc.vector.reciprocal(rs[:], sp[:])
            den = den_pool.tile([128, SQ], F32, tag="den")
            nc.gpsimd.partition_broadcast(den[:], rs[:])
            nc.vector.tensor_tensor(
                attn_sb[:, h * SQ : (h + 1) * SQ], op[:], den[:], ALU.mult
            )

        # ---------- phase 4: o_proj ----------
        attn_es.close()
        with (
            tc.tile_pool(name="o_ps", bufs=2, space="PSUM") as o_ps,
            tc.tile_pool(name="o_sb", bufs=3) as o_sb_pool,
        ):
            if with_bias_o:
                bo_sb = const_pool.tile([1, E], F32, tag="bo")
                nc.sync.dma_start(bo_sb[:], t["bod"])
                bo_b = const_pool.tile([128, E], F32, tag="bo_b")
                nc.gpsimd.partition_broadcast(bo_b[:], bo_sb[:])
            for et in range(4):
                for sqc in range(SQ // 128):
                    ps = o_ps.tile([128, 512], F32, tag="ops")
                    for hd in range(HQ):
                        nc.tensor.matmul(
                            ps[:],
                            attn_sb[:, hd * SQ + sqc * 128 : hd * SQ + (sqc + 1) * 128],
                            woview[:, hd, et * 512 : (et + 1) * 512],
                            start=(hd == 0),
                            stop=(hd == HQ - 1),
                        )
                    ot = o_sb_pool.tile([128, 512], F32, tag="osb")
                    if with_bias_o:
                        nc.vector.tensor_tensor(
                            ot[:], ps[:], bo_b[:, et * 512 : (et + 1) * 512], ALU.add
                        )
                    else:
                        nc.scalar.copy(ot[:], ps[:])
                    nc.sync.dma_start(
                        out[sqc * 128 : (sqc + 1) * 128, et * 512 : (et + 1) * 512],
                        ot[:],
                    )


RUN_KWARGS = {}


def kernel(x, sin, cos, Wq, bq, Wk, bk, Wv, bv, Wo, bo, sinks):
    x = np.asarray(x, dtype=np.float32)
    sin = np.asarray(sin, dtype=np.float32)
    cos = np.asarray(cos, dtype=np.float32)
    sinks = np.asarray(sinks, dtype=np.float32)
    with_bias_qkv = bool(np.any(bq) or np.any(bk) or np.any(bv))
    with_bias_o = bool(np.any(bo))

    key = (sinks.tobytes(), with_bias_qkv, with_bias_o)
    if key not in _CACHE:
        _CACHE[key] = _build(sinks, with_bias_qkv, with_bias_o)
    nc = _CACHE[key]

    def tile_w(W, H):
        # [E, H*128] -> [128, H*EC*128] with free index (h, c, n)
        W = np.asarray(W, dtype=np.float32)
        return np.ascontiguousarray(
            W.reshape(EC, 128, H, 128).transpose(1, 2, 0, 3).reshape(128, H * EC * 128)
        ).astype(bfloat16)

    wq_t = tile_w(Wq, HQ)
    wk_t = tile_w(Wk, HKV)
    wv_t = tile_w(Wv, HKV)
    # Wo [HQ*D, E] -> [128, HQ*E] with free index (hd, e)
    wo_t = np.ascontiguousarray(
        np.asarray(Wo, np.float32).reshape(HQ, 128, E).transpose(1, 0, 2).reshape(128, HQ * E)
    ).astype(bfloat16)

    in_maps = []
    for dev in range(NDEV):
        b, i = divmod(dev, DPB)
        sl = slice(SQ * i, SQ * (i + 1))
        xs = x[b, sl, :]  # [SQ, E]
        xT_t = np.ascontiguousarray(
            xs.T.reshape(EC, 128, SQ).transpose(1, 0, 2).reshape(128, EC * SQ)
        ).astype(bfloat16)
        m = {
            "xT": xT_t,
            "wq": wq_t,
            "wk": wk_t,
            "wv": wv_t,
            "wo": wo_t,
            "cosT": np.ascontiguousarray(cos[b, sl, :].T),
            "sinT": np.ascontiguousarray(sin[b, sl, :].T),
        }
        if with_bias_qkv:
            m["bqd"] = np.ascontiguousarray(np.asarray(bq, np.float32).reshape(HQ, D).T)
            m["bkd"] = np.ascontiguousarray(np.asarray(bk, np.float32).reshape(HKV, D).T)
            m["bvd"] = np.ascontiguousarray(np.asarray(bv, np.float32).reshape(HKV, D).T)
        if with_bias_o:
            m["bod"] = np.asarray(bo, np.float32).reshape(1, E)
        in_maps.append(m)

    res = run_bass_kernel_spmd(nc, in_maps, list(range(NDEV)), **RUN_KWARGS)
    kernel.last_result = res

    out = np.empty((B, S, E), dtype=np.float32)
    for dev in range(NDEV):
        b, i = divmod(dev, DPB)
        out[b, SQ * i : SQ * (i + 1), :] = res.results[dev]["out"]
    return out
